# revision 4
# baseline (speedup 1.0000x reference)
"""MultiLabelSoftMarginLoss (logits=True path) on 8 Trainium2 NeuronCores.

Math (per sample b, C classes, K labels t_bk, ls = log_sigmoid):
  pos_mean_b = (1/K) sum_k ls(g_bk),  g_bk = x[b, t_bk]
  neg_mean_b = [sum_c ls(-x_bc) - sum_{unique labels u} ls(-x_bu)] / (C - n_uniq_b)
  loss = -mean_b(pos_mean_b + neg_mean_b)

Bulk term: ls(-x) = ln sigmoid(-x) and sum_c ln s_c = ln prod s_c, so each
chunk computes s = sigmoid(-x) on the ACT engine (bf16 out), multiplies
groups of 8 together with three unit-stride fold multiplies on the DVE,
and deferred Ln+row-accumulate windows recover sum_c ls(-x_c) while
touching only 1/8 of the elements.

Schedule: the stream is DMA-roofline-bound (~136us of HBM traffic);
ACT and DVE both have slack, so BOTH engines' instruction order is
pinned explicitly (add_dep chains). Anything that transitively depends
on the indirect gathers (done ~45/67us) or on the Ln clusters is placed
late enough in each chain to never head-of-line block the stream — the
scheduler otherwise hoists e.g. gather reduces to the front of the DVE
program, stalling folds -> sigmoid scratch WAR -> ACT -> DMA. sum_k g
is accumulated on ACT via an Identity pass (present in every table) so
the DVE never touches raw g. Ln work sits in three compact windows
(blk0 full early in blk1's stream, blk1 cols 0..J mid-stream, short
tail), each one Sigmoid<->Ln table-load pair. Chunk widths taper at
the block end to shorten the exposed post-stream chain.

Positive/dedup correction: gather g via per-column indirect DMAs
(issued first, spread over 4 SWDGE queues, fully overlapped), then
ls(g) = g + ln sigmoid(-g) and first-occurrence dedup weights via
pairwise label compares on tiny [128, K] tiles.

Data-parallel: 2048 rows sharded 256/core; host sums 8x256 per-row
losses and negates.
"""

import numpy as np

import concourse.bacc as bacc
import concourse.bass as bass
import concourse.mybir as mybir
import concourse.tile as tile
from concourse.bass_utils import run_bass_kernel_spmd
from concourse.tile_rust import add_dep_helper

B, C, K = 2048, 50257, 20
NCORES = 8
RPC = B // NCORES  # rows per core
P = 128
NBLK = RPC // P  # row blocks of 128 partitions per core
CHUNK = 3072
# taper the end of each block so the exposed post-stream ACT tail is short;
# last chunk pads 513 -> 520 so the three fold-halvings stay even
WIDTHS = [CHUNK] * 15 + [2048, 1616, 513]
assert sum(WIDTHS) == C
NW = len(WIDTHS)
WPAD = [-(-w // 8) * 8 for w in WIDTHS]
PTOFF = [0]
for w in WPAD:
    PTOFF.append(PTOFF[-1] + w // 8)
PROD_COLS = PTOFF[-1]  # 6283

# anchor points (chunk index within blk1's stream)
B0_CI = 2    # blk0's Ln cluster goes after blk1 chunk 2's sigmoid
FIN0_CI = 8  # blk0's finish chain (needs T0 from that cluster)
MID_J = 10   # blk1's mid Ln window after chunk J covers cols of chunks 0..J
FIN1_CI = 13  # blk1's gather-side sums (need lnsgn1 from mid window)

F32 = mybir.dt.float32
BF16 = mybir.dt.bfloat16
I32 = mybir.dt.int32
AF = mybir.ActivationFunctionType
ALU = mybir.AluOpType
AX = mybir.AxisListType

_CACHE = {}


class Chain:
    """Explicit per-engine instruction-order chain."""

    def __init__(self, reason):
        self.last = None
        self.reason = reason

    def add(self, inst):
        if self.last is not None:
            add_dep_helper(
                inst.ins, self.last.ins, sync=False, reason=self.reason
            )
        self.last = inst
        return inst


def _build():
    nc = bacc.Bacc(
        "TRN2", target_bir_lowering=False, debug=False, num_devices=NCORES,
        num_swdge_queues=4,
    )
    x = nc.dram_tensor("x", [RPC, C], F32, kind="ExternalInput").ap()
    t = nc.dram_tensor("t", [RPC, K], I32, kind="ExternalInput").ap()
    o = nc.dram_tensor("o", [RPC, K], I32, kind="ExternalInput").ap()
    out = nc.dram_tensor("out", [NBLK, P], F32, kind="ExternalOutput").ap()

    with tile.TileContext(nc) as tc:
        with (
            tc.tile_pool(name="xpool", bufs=8) as xpool,
            tc.tile_pool(name="spool", bufs=3) as spool,
            tc.tile_pool(name="scr", bufs=2) as scr,
            tc.tile_pool(name="small", bufs=2) as small,
        ):
            act = Chain("act order")
            dve = Chain("dve order")

            # ---- phase 0: gathers up front so they overlap streaming ----
            tts, gs = [], []
            for blk in range(NBLK):
                rows = slice(blk * P, (blk + 1) * P)
                tt = small.tile([P, K], I32, tag="tt")
                tts.append(tt)
                nc.sync.dma_start(out=tt[:], in_=t[rows, :])
                # flat offsets row*C + t are precomputed on the host
                offs = small.tile([P, K], I32, tag="offs")
                nc.sync.dma_start(out=offs[:], in_=o[rows, :])
                g = small.tile([P, K], F32, tag="g")
                gs.append(g)
                for k in range(K):
                    inst = nc.gpsimd.indirect_dma_start(
                        out=g[:, k : k + 1],
                        out_offset=None,
                        in_=x[:, :],
                        in_offset=bass.IndirectOffsetOnAxis(
                            ap=offs[:, k : k + 1], axis=1
                        ),
                    )
                    qi = (blk * K + k) % 4
                    if qi:
                        inst.ins.queue = f"qPoolDynamic{qi}"

            # per-block persistent tiles
            pt_alls = [
                small.tile([P, PROD_COLS], BF16, tag="pt_all", name=f"pt{b}")
                for b in range(NBLK)
            ]
            sgns = [
                small.tile([P, K], F32, tag="sgn", name=f"sgn{b}")
                for b in range(NBLK)
            ]
            lnsgns = [
                small.tile([P, K], F32, tag="lnsgn", name=f"lnsgn{b}")
                for b in range(NBLK)
            ]
            lnsgn_sums = [
                small.tile([P, 1], F32, tag="lnsgn_sum", name=f"lnsgn_sum{b}")
                for b in range(NBLK)
            ]
            g_sums = [
                small.tile([P, 1], F32, tag="g_sum", name=f"g_sum{b}")
                for b in range(NBLK)
            ]
            gcols = [
                small.tile([P, K], F32, tag="gcol", name=f"gcol{b}")
                for b in range(NBLK)
            ]
            Ts = [
                small.tile([P, 1], F32, tag="T", name=f"T{b}")
                for b in range(NBLK)
            ]
            T1b = small.tile([P, 1], F32, tag="T1b", name="T1b")

            # ---- dedup weights: DVE-only, inputs land at ~5us; left
            # unpinned so the scheduler front-loads them into DVE idle ----
            ws, recips = [], []
            for blk in range(NBLK):
                tf = small.tile([P, K], F32, tag="tf")
                nc.vector.tensor_copy(out=tf[:], in_=tts[blk][:])
                dup = small.tile([P, K], F32, tag="dup")
                nc.vector.memset(dup[:, 0:1], 0.0)
                eq = small.tile([P, K], F32, tag="eq")
                for k in range(1, K):
                    nc.vector.tensor_scalar(
                        out=eq[:, :k], in0=tf[:, :k],
                        scalar1=tf[:, k : k + 1], scalar2=None,
                        op0=ALU.is_equal,
                    )
                    nc.vector.reduce_max(
                        out=dup[:, k : k + 1], in_=eq[:, :k], axis=AX.X
                    )
                w = small.tile([P, K], F32, tag="w", name=f"w{blk}")
                nc.vector.tensor_scalar(
                    out=w[:], in0=dup[:], scalar1=-1.0, scalar2=1.0,
                    op0=ALU.mult, op1=ALU.add,
                )
                ws.append(w)
                u = small.tile([P, 1], F32, tag="u")
                nc.vector.reduce_sum(out=u[:], in_=w[:], axis=AX.X)
                denom = small.tile([P, 1], F32, tag="denom")
                nc.vector.tensor_scalar(
                    out=denom[:], in0=u[:], scalar1=-1.0, scalar2=float(C),
                    op0=ALU.mult, op1=ALU.add,
                )
                recip = small.tile([P, 1], F32, tag="recip", name=f"recip{blk}")
                nc.vector.reciprocal(out=recip[:], in_=denom[:])
                recips.append(recip)

            def fold_products(s, width, pt_all, pt_off):
                """s[:, :width] (bf16) -> width/8 group products in
                pt_all[:, pt_off:pt_off+width//8]; DVE-chain pinned."""
                w2, w4, w8 = width // 2, width // 4, width // 8
                h1 = scr.tile([P, CHUNK // 2], BF16, tag="h1")
                dve.add(nc.vector.tensor_tensor(
                    out=h1[:, :w2], in0=s[:, :w2], in1=s[:, w2:width],
                    op=ALU.mult,
                ))
                h2 = scr.tile([P, CHUNK // 4], BF16, tag="h2")
                dve.add(nc.vector.tensor_tensor(
                    out=h2[:, :w4], in0=h1[:, :w4], in1=h1[:, w4:w2],
                    op=ALU.mult,
                ))
                dve.add(nc.vector.tensor_tensor(
                    out=pt_all[:, pt_off : pt_off + w8],
                    in0=h2[:, :w8], in1=h2[:, w8:w4], op=ALU.mult,
                ))

            def emit_ln_cluster(blk, lo, hi, acc):
                """[Sigmoid(-g), Ln small w/ accum, Identity g w/ accum,
                Ln over pt cols lo:hi w/ accum into acc] on the ACT chain."""
                act.add(nc.scalar.activation(
                    sgns[blk][:], gs[blk][:], AF.Sigmoid, scale=-1.0
                ))
                act.add(nc.scalar.activation(
                    lnsgns[blk][:], sgns[blk][:], AF.Ln,
                    accum_out=lnsgn_sums[blk][:],
                ))
                act.add(nc.scalar.activation(
                    gcols[blk][:], gs[blk][:], AF.Identity,
                    accum_out=g_sums[blk][:],
                ))
                act.add(nc.scalar.activation(
                    pt_alls[blk][:, lo:hi], pt_alls[blk][:, lo:hi],
                    AF.Ln, accum_out=acc[:],
                ))

            def emit_gather_sums(blk):
                """dsum, posm on the DVE chain (needs lnsgn/lnsgn_sum/
                g_sum from the block's Ln cluster)."""
                wl = small.tile([P, K], F32, tag="wl")
                dve.add(nc.vector.tensor_tensor(
                    out=wl[:], in0=ws[blk][:], in1=lnsgns[blk][:],
                    op=ALU.mult,
                ))
                dsum = small.tile([P, 1], F32, tag="dsum", name=f"dsum{blk}")
                dve.add(nc.vector.reduce_sum(out=dsum[:], in_=wl[:], axis=AX.X))
                posm = small.tile([P, 1], F32, tag="posm", name=f"posm{blk}")
                dve.add(nc.vector.tensor_add(
                    out=posm[:], in0=g_sums[blk][:], in1=lnsgn_sums[blk][:]
                ))
                dve.add(nc.vector.tensor_scalar(
                    out=posm[:], in0=posm[:], scalar1=1.0 / K, scalar2=None,
                    op0=ALU.mult,
                ))
                return dsum, posm

            def emit_finish(blk, dsum, posm):
                """negm + loss + out DMA on the DVE chain (needs T)."""
                negm = small.tile([P, 1], F32, tag="negm")
                dve.add(nc.vector.tensor_sub(
                    out=negm[:], in0=Ts[blk][:], in1=dsum[:]
                ))
                dve.add(nc.vector.tensor_mul(
                    out=negm[:], in0=negm[:], in1=recips[blk][:]
                ))
                loss = small.tile([P, 1], F32, tag="loss")
                dve.add(nc.vector.tensor_add(
                    out=loss[:], in0=posm[:], in1=negm[:]
                ))
                nc.sync.dma_start(out=out[blk, :, None], in_=loss[:])

            # ---- streaming sigmoid+fold pass ----
            side = {}
            for blk in range(NBLK):
                rows = slice(blk * P, (blk + 1) * P)
                pt_all = pt_alls[blk]
                c0 = 0
                for ci, cw in enumerate(WIDTHS):
                    cwp = WPAD[ci]
                    xt = xpool.tile([P, CHUNK], F32, tag="xt")
                    if cw != cwp:
                        # pad -> sigmoid(30)=1.0 -> neutral for products
                        dve.add(nc.vector.memset(xt[:, cw:cwp], -30.0))
                    nc.sync.dma_start(out=xt[:, :cw], in_=x[rows, c0 : c0 + cw])
                    s = spool.tile([P, CHUNK], BF16, tag="s")
                    act.add(nc.scalar.activation(
                        s[:, :cwp], xt[:, :cwp], AF.Sigmoid, scale=-1.0
                    ))
                    fold_products(s, cwp, pt_all, PTOFF[ci])
                    c0 += cw

                    if blk == 1 and ci == B0_CI:
                        emit_ln_cluster(0, 0, PROD_COLS, Ts[0])
                    elif blk == 1 and ci == FIN0_CI:
                        side["d0"], side["p0"] = emit_gather_sums(0)
                        emit_finish(0, side["d0"], side["p0"])
                    elif blk == 1 and ci == MID_J:
                        emit_ln_cluster(1, 0, PTOFF[MID_J + 1], Ts[1])
                    elif blk == 1 and ci == FIN1_CI:
                        side["d1"], side["p1"] = emit_gather_sums(1)

            # tail: Ln over blk1's remaining product columns, then combine
            act.add(nc.scalar.activation(
                pt_alls[1][:, PTOFF[MID_J + 1] :],
                pt_alls[1][:, PTOFF[MID_J + 1] :],
                AF.Ln, accum_out=T1b[:],
            ))
            dve.add(nc.vector.tensor_add(
                out=Ts[1][:], in0=Ts[1][:], in1=T1b[:]
            ))
            emit_finish(1, side["d1"], side["p1"])

    nc.compile()
    return nc


def kernel(inputs: np.ndarray, targets: np.ndarray, _trace: bool = False):
    inputs = np.ascontiguousarray(inputs, dtype=np.float32)
    targets = np.ascontiguousarray(targets, dtype=np.int32)
    assert inputs.shape == (B, C) and targets.shape == (B, K)

    if "nc" not in _CACHE:
        _CACHE["nc"] = _build()
    nc = _CACHE["nc"]

    offs_np = targets.astype(np.int64) + (np.arange(B, dtype=np.int64) % RPC)[
        :, None
    ] * C
    offs_np = offs_np.astype(np.int32)
    in_maps = [
        {
            "x": inputs[i * RPC : (i + 1) * RPC],
            "t": targets[i * RPC : (i + 1) * RPC],
            "o": offs_np[i * RPC : (i + 1) * RPC],
        }
        for i in range(NCORES)
    ]
    res = run_bass_kernel_spmd(
        nc, in_maps, core_ids=list(range(NCORES)), trace=_trace
    )
    _CACHE["last_results"] = res

    per_row = np.concatenate(
        [res.results[i]["out"].reshape(-1) for i in range(NCORES)]
    )
    return np.float32(-np.mean(per_row, dtype=np.float64))


# revision 5
# speedup vs baseline: 1.0555x; 1.0555x over previous
"""MultiLabelSoftMarginLoss (logits=True path) on 8 Trainium2 NeuronCores.

Math (per sample b, C classes, K labels t_bk, ls = log_sigmoid):
  pos_mean_b = (1/K) sum_k ls(g_bk),  g_bk = x[b, t_bk]
  neg_mean_b = [sum_c ls(-x_bc) - sum_{unique labels u} ls(-x_bu)] / (C - n_uniq_b)
  loss = -mean_b(pos_mean_b + neg_mean_b)

Bulk term: ls(-x) = ln sigmoid(-x) and sum_c ln s_c = ln prod s_c, so each
chunk computes s = sigmoid(-x) on the ACT engine (bf16 out), multiplies
groups of 8 together with three unit-stride fold multiplies on the DVE,
and deferred Ln+row-accumulate windows recover sum_c ls(-x_c) while
touching only 1/8 of the elements.

Schedule: the stream is DMA-roofline-bound (~136us of HBM traffic);
ACT and DVE both have slack, so BOTH engines' instruction order is
pinned explicitly (add_dep chains). Anything that transitively depends
on the indirect gathers (done ~45/67us) or on the Ln clusters is placed
late enough in each chain to never head-of-line block the stream — the
scheduler otherwise hoists e.g. gather reduces to the front of the DVE
program, stalling folds -> sigmoid scratch WAR -> ACT -> DMA. sum_k g
is accumulated on ACT via an Identity pass (present in every table) so
the DVE never touches raw g. Ln work sits in three compact windows
(blk0 full early in blk1's stream, blk1 cols 0..J mid-stream, short
tail), each one Sigmoid<->Ln table-load pair. Chunk widths taper at
the block end to shorten the exposed post-stream chain.

Positive/dedup correction: gather g via per-column indirect DMAs
(issued first, spread over 4 SWDGE queues, fully overlapped), then
ls(g) = g + ln sigmoid(-g) and first-occurrence dedup weights via
pairwise label compares on tiny [128, K] tiles.

Data-parallel: 2048 rows sharded 256/core; host sums 8x256 per-row
losses and negates.
"""

import numpy as np

import concourse.bacc as bacc
import concourse.bass as bass
import concourse.mybir as mybir
import concourse.tile as tile
from concourse.bass_utils import run_bass_kernel_spmd
from concourse.tile_rust import add_dep_helper

B, C, K = 2048, 50257, 20
NCORES = 8
RPC = B // NCORES  # rows per core
P = 128
NBLK = RPC // P  # row blocks of 128 partitions per core
CHUNK = 3072
# taper the end of each block so the exposed post-stream ACT tail is short;
# last chunk pads 513 -> 520 so the three fold-halvings stay even
WIDTHS = [CHUNK] * 15 + [2048, 1616, 513]
assert sum(WIDTHS) == C
NW = len(WIDTHS)
WPAD = [-(-w // 8) * 8 for w in WIDTHS]
PTOFF = [0]
for w in WPAD:
    PTOFF.append(PTOFF[-1] + w // 8)
PROD_COLS = PTOFF[-1]  # 6283

# anchor points (chunk index within blk1's stream)
B0_CI = 2    # blk0's Ln cluster goes after blk1 chunk 2's sigmoid
FIN0_CI = 8  # blk0's finish chain (needs T0 from that cluster)
MID_J = 12   # blk1's mid Ln window after chunk J covers cols of chunks 0..J
FIN1_CI = 14  # blk1's gather-side sums (need lnsgn1 from mid window)

F32 = mybir.dt.float32
BF16 = mybir.dt.bfloat16
I32 = mybir.dt.int32
AF = mybir.ActivationFunctionType
ALU = mybir.AluOpType
AX = mybir.AxisListType

_CACHE = {}


class Chain:
    """Explicit per-engine instruction-order chain."""

    def __init__(self, reason):
        self.last = None
        self.reason = reason

    def add(self, inst):
        if self.last is not None:
            add_dep_helper(
                inst.ins, self.last.ins, sync=False, reason=self.reason
            )
        self.last = inst
        return inst


def _build():
    nc = bacc.Bacc(
        "TRN2", target_bir_lowering=False, debug=False, num_devices=NCORES,
        num_swdge_queues=4,
    )
    x = nc.dram_tensor("x", [RPC, C], F32, kind="ExternalInput").ap()
    t = nc.dram_tensor("t", [RPC, K], I32, kind="ExternalInput").ap()
    o = nc.dram_tensor("o", [RPC, K], I32, kind="ExternalInput").ap()
    out = nc.dram_tensor("out", [NBLK, P], F32, kind="ExternalOutput").ap()

    with tile.TileContext(nc) as tc:
        with (
            tc.tile_pool(name="xpool", bufs=10) as xpool,
            tc.tile_pool(name="spool", bufs=3) as spool,
            tc.tile_pool(name="scr", bufs=2) as scr,
            tc.tile_pool(name="small", bufs=2) as small,
        ):
            act = Chain("act order")
            dve = Chain("dve order")

            # ---- phase 0: gathers up front so they overlap streaming ----
            tts, gs = [], []
            for blk in range(NBLK):
                rows = slice(blk * P, (blk + 1) * P)
                tt = small.tile([P, K], I32, tag="tt")
                tts.append(tt)
                nc.sync.dma_start(out=tt[:], in_=t[rows, :])
                # flat offsets row*C + t are precomputed on the host
                offs = small.tile([P, K], I32, tag="offs")
                nc.sync.dma_start(out=offs[:], in_=o[rows, :])
                g = small.tile([P, K], F32, tag="g")
                gs.append(g)
                for k in range(K):
                    inst = nc.gpsimd.indirect_dma_start(
                        out=g[:, k : k + 1],
                        out_offset=None,
                        in_=x[:, :],
                        in_offset=bass.IndirectOffsetOnAxis(
                            ap=offs[:, k : k + 1], axis=1
                        ),
                    )
                    qi = (blk * K + k) % 4
                    if qi:
                        inst.ins.queue = f"qPoolDynamic{qi}"

            # per-block persistent tiles
            pt_alls = [
                small.tile([P, PROD_COLS], BF16, tag="pt_all", name=f"pt{b}")
                for b in range(NBLK)
            ]
            sgns = [
                small.tile([P, K], F32, tag="sgn", name=f"sgn{b}")
                for b in range(NBLK)
            ]
            lnsgns = [
                small.tile([P, K], F32, tag="lnsgn", name=f"lnsgn{b}")
                for b in range(NBLK)
            ]
            lnsgn_sums = [
                small.tile([P, 1], F32, tag="lnsgn_sum", name=f"lnsgn_sum{b}")
                for b in range(NBLK)
            ]
            g_sums = [
                small.tile([P, 1], F32, tag="g_sum", name=f"g_sum{b}")
                for b in range(NBLK)
            ]
            gcols = [
                small.tile([P, K], F32, tag="gcol", name=f"gcol{b}")
                for b in range(NBLK)
            ]
            Ts = [
                small.tile([P, 1], F32, tag="T", name=f"T{b}")
                for b in range(NBLK)
            ]
            T1b = small.tile([P, 1], F32, tag="T1b", name="T1b")

            # ---- dedup weights: DVE-only, inputs land at ~5us; left
            # unpinned so the scheduler front-loads them into DVE idle ----
            ws, recips = [], []
            for blk in range(NBLK):
                tf = small.tile([P, K], F32, tag="tf")
                nc.vector.tensor_copy(out=tf[:], in_=tts[blk][:])
                dup = small.tile([P, K], F32, tag="dup")
                nc.vector.memset(dup[:, 0:1], 0.0)
                eq = small.tile([P, K], F32, tag="eq")
                for k in range(1, K):
                    nc.vector.tensor_scalar(
                        out=eq[:, :k], in0=tf[:, :k],
                        scalar1=tf[:, k : k + 1], scalar2=None,
                        op0=ALU.is_equal,
                    )
                    nc.vector.reduce_max(
                        out=dup[:, k : k + 1], in_=eq[:, :k], axis=AX.X
                    )
                w = small.tile([P, K], F32, tag="w", name=f"w{blk}")
                nc.vector.tensor_scalar(
                    out=w[:], in0=dup[:], scalar1=-1.0, scalar2=1.0,
                    op0=ALU.mult, op1=ALU.add,
                )
                ws.append(w)
                u = small.tile([P, 1], F32, tag="u")
                nc.vector.reduce_sum(out=u[:], in_=w[:], axis=AX.X)
                denom = small.tile([P, 1], F32, tag="denom")
                nc.vector.tensor_scalar(
                    out=denom[:], in0=u[:], scalar1=-1.0, scalar2=float(C),
                    op0=ALU.mult, op1=ALU.add,
                )
                recip = small.tile([P, 1], F32, tag="recip", name=f"recip{blk}")
                nc.vector.reciprocal(out=recip[:], in_=denom[:])
                recips.append(recip)

            def fold_products(s, width, pt_all, pt_off):
                """s[:, :width] (bf16) -> width/8 group products in
                pt_all[:, pt_off:pt_off+width//8]; DVE-chain pinned."""
                w2, w4, w8 = width // 2, width // 4, width // 8
                h1 = scr.tile([P, CHUNK // 2], BF16, tag="h1")
                dve.add(nc.vector.tensor_tensor(
                    out=h1[:, :w2], in0=s[:, :w2], in1=s[:, w2:width],
                    op=ALU.mult,
                ))
                h2 = scr.tile([P, CHUNK // 4], BF16, tag="h2")
                dve.add(nc.vector.tensor_tensor(
                    out=h2[:, :w4], in0=h1[:, :w4], in1=h1[:, w4:w2],
                    op=ALU.mult,
                ))
                dve.add(nc.vector.tensor_tensor(
                    out=pt_all[:, pt_off : pt_off + w8],
                    in0=h2[:, :w8], in1=h2[:, w8:w4], op=ALU.mult,
                ))

            def emit_ln_cluster(blk, lo, hi, acc):
                """[Sigmoid(-g), Ln small w/ accum, Identity g w/ accum,
                Ln over pt cols lo:hi w/ accum into acc] on the ACT chain."""
                act.add(nc.scalar.activation(
                    sgns[blk][:], gs[blk][:], AF.Sigmoid, scale=-1.0
                ))
                act.add(nc.scalar.activation(
                    lnsgns[blk][:], sgns[blk][:], AF.Ln,
                    accum_out=lnsgn_sums[blk][:],
                ))
                act.add(nc.scalar.activation(
                    gcols[blk][:], gs[blk][:], AF.Identity,
                    accum_out=g_sums[blk][:],
                ))
                act.add(nc.scalar.activation(
                    pt_alls[blk][:, lo:hi], pt_alls[blk][:, lo:hi],
                    AF.Ln, accum_out=acc[:],
                ))

            def emit_gather_sums(blk):
                """dsum, posm on the DVE chain (needs lnsgn/lnsgn_sum/
                g_sum from the block's Ln cluster)."""
                wl = small.tile([P, K], F32, tag="wl")
                dve.add(nc.vector.tensor_tensor(
                    out=wl[:], in0=ws[blk][:], in1=lnsgns[blk][:],
                    op=ALU.mult,
                ))
                dsum = small.tile([P, 1], F32, tag="dsum", name=f"dsum{blk}")
                dve.add(nc.vector.reduce_sum(out=dsum[:], in_=wl[:], axis=AX.X))
                posm = small.tile([P, 1], F32, tag="posm", name=f"posm{blk}")
                dve.add(nc.vector.tensor_add(
                    out=posm[:], in0=g_sums[blk][:], in1=lnsgn_sums[blk][:]
                ))
                dve.add(nc.vector.tensor_scalar(
                    out=posm[:], in0=posm[:], scalar1=1.0 / K, scalar2=None,
                    op0=ALU.mult,
                ))
                return dsum, posm

            def emit_finish(blk, dsum, posm):
                """negm + loss + out DMA on the DVE chain (needs T)."""
                negm = small.tile([P, 1], F32, tag="negm")
                dve.add(nc.vector.tensor_sub(
                    out=negm[:], in0=Ts[blk][:], in1=dsum[:]
                ))
                dve.add(nc.vector.tensor_mul(
                    out=negm[:], in0=negm[:], in1=recips[blk][:]
                ))
                loss = small.tile([P, 1], F32, tag="loss")
                dve.add(nc.vector.tensor_add(
                    out=loss[:], in0=posm[:], in1=negm[:]
                ))
                nc.scalar.dma_start(out=out[blk, :, None], in_=loss[:])

            # ---- streaming sigmoid+fold pass ----
            side = {}
            for blk in range(NBLK):
                rows = slice(blk * P, (blk + 1) * P)
                pt_all = pt_alls[blk]
                c0 = 0
                for ci, cw in enumerate(WIDTHS):
                    cwp = WPAD[ci]
                    xt = xpool.tile([P, CHUNK], F32, tag="xt")
                    if cw != cwp:
                        # pad -> sigmoid(30)=1.0 -> neutral for products
                        dve.add(nc.vector.memset(xt[:, cw:cwp], -30.0))
                    nc.sync.dma_start(out=xt[:, :cw], in_=x[rows, c0 : c0 + cw])
                    s = spool.tile([P, CHUNK], BF16, tag="s")
                    act.add(nc.scalar.activation(
                        s[:, :cwp], xt[:, :cwp], AF.Sigmoid, scale=-1.0
                    ))
                    fold_products(s, cwp, pt_all, PTOFF[ci])
                    c0 += cw

                    if blk == 1 and ci == B0_CI:
                        emit_ln_cluster(0, 0, PROD_COLS, Ts[0])
                    elif blk == 1 and ci == FIN0_CI:
                        side["d0"], side["p0"] = emit_gather_sums(0)
                        emit_finish(0, side["d0"], side["p0"])
                    elif blk == 1 and ci == MID_J:
                        emit_ln_cluster(1, 0, PTOFF[MID_J + 1], Ts[1])
                    elif blk == 1 and ci == FIN1_CI:
                        side["d1"], side["p1"] = emit_gather_sums(1)

            # tail: Ln over blk1's remaining product columns, then combine
            act.add(nc.scalar.activation(
                pt_alls[1][:, PTOFF[MID_J + 1] :],
                pt_alls[1][:, PTOFF[MID_J + 1] :],
                AF.Ln, accum_out=T1b[:],
            ))
            dve.add(nc.vector.tensor_add(
                out=Ts[1][:], in0=Ts[1][:], in1=T1b[:]
            ))
            emit_finish(1, side["d1"], side["p1"])

    nc.compile()
    return nc


def kernel(inputs: np.ndarray, targets: np.ndarray, _trace: bool = False):
    inputs = np.ascontiguousarray(inputs, dtype=np.float32)
    targets = np.ascontiguousarray(targets, dtype=np.int32)
    assert inputs.shape == (B, C) and targets.shape == (B, K)

    if "nc" not in _CACHE:
        _CACHE["nc"] = _build()
    nc = _CACHE["nc"]

    offs_np = targets.astype(np.int64) + (np.arange(B, dtype=np.int64) % RPC)[
        :, None
    ] * C
    offs_np = offs_np.astype(np.int32)
    in_maps = [
        {
            "x": inputs[i * RPC : (i + 1) * RPC],
            "t": targets[i * RPC : (i + 1) * RPC],
            "o": offs_np[i * RPC : (i + 1) * RPC],
        }
        for i in range(NCORES)
    ]
    res = run_bass_kernel_spmd(
        nc, in_maps, core_ids=list(range(NCORES)), trace=_trace
    )
    _CACHE["last_results"] = res

    per_row = np.concatenate(
        [res.results[i]["out"].reshape(-1) for i in range(NCORES)]
    )
    return np.float32(-np.mean(per_row, dtype=np.float64))
